# revision 7
# baseline (speedup 1.0000x reference)
"""LMU kernel for Trainium2, 8-core data-parallel.

Math (per batch b, with x[b] in [D, L] layout):
  u[b]    = relu(W_u @ x[b] + b_u)                              [1, L]
  m[b]    = H @ Toep(u[b])        (causal conv via Toeplitz)    [D, L]
  h[b]    = relu(W_h[:, :D] @ m[b] + W_h[:, D:] @ x[b] + b_h)   [D, L]
  y[b]    = BN(conv_w @ h[b] + conv_b)                          [D, L]

Device-side folds (host precomputes, O(params) only):
  F      = (W_h[:, :D] @ H).T, row-flipped  -> single K=128 contraction
           against the (flipped) Toeplitz of u
  C'     = (inv * conv_w).T, bias' = (conv_b - mean) * inv + beta   (BN fold)

All matmul operands are bf16 (host-cast), so LDWEIGHTS hides fully under
the 1 col/cycle stream and no on-device casts are needed anywhere.
Batch dim sharded 8 ways; params replicated.
"""

import os
import numpy as np
import ml_dtypes

import concourse.bass as bass
import concourse.mybir as mybir
from concourse import bacc
from concourse.tile import TileContext
from concourse.bass_utils import run_bass_kernel_spmd

B, D, L = 256, 768, 128
NCORES = 8
BPC = B // NCORES          # batches per core
NB = 4                     # batches per column block
NCB = BPC // NB            # column blocks per core
NCOL = NB * L              # 512 columns per block
KC = D // 128              # 6 chunks of 128 over the D dim
THETA = 128.0
BN_EPS = 1e-5

TRACE = False
LAST_EXEC_NS = None

_H_CACHE = None
_NC_CACHE = None


def _impulse_response():
    """Replicates the reference's H = impulse response [D, L], on CPU."""
    global _H_CACHE
    if _H_CACHE is not None:
        return _H_CACHE
    import jax
    import jax.numpy as jnp
    from jax.scipy.linalg import expm

    cpu = jax.devices("cpu")[0]
    with jax.default_device(cpu):
        Q = np.arange(D, dtype=np.float32)
        R = ((2.0 * Q + 1.0) / THETA)[:, None]
        i, j = np.meshgrid(Q, Q, indexing="ij")
        A = (np.where(i < j, -1.0, (-1.0) ** (i - j + 1)).astype(np.float32)) * R
        Bm = (((-1.0) ** Q)[:, None]).astype(np.float32) * R
        Maug = np.zeros((D + 1, D + 1), dtype=np.float32)
        Maug[:D, :D] = A
        Maug[:D, D:] = Bm
        E = expm(jnp.asarray(Maug))
        Ad = E[:D, :D]
        Bd = E[:D, D:]

        def step(Apow, _):
            return Ad @ Apow, (Apow @ Bd)[:, 0]

        _, H = jax.lax.scan(step, jnp.eye(D, dtype=jnp.float32), None, length=L)
        _H_CACHE = np.asarray(H).T.astype(np.float32)  # [D, L]
    return _H_CACHE


def _build_nc():
    """Builds the (static) 8-core SPMD Bass program."""
    f32 = mybir.dt.float32
    bf16 = mybir.dt.bfloat16
    nc = bacc.Bacc("TRN2", target_bir_lowering=False, debug=False, num_devices=NCORES)

    x_d = nc.dram_tensor("x", [BPC, D, L], bf16, kind="ExternalInput").ap()
    whxT_d = nc.dram_tensor("whxT", [D, D], bf16, kind="ExternalInput").ap()
    ct_d = nc.dram_tensor("ct", [D, D], bf16, kind="ExternalInput").ap()
    f_d = nc.dram_tensor("fmat", [L, D], bf16, kind="ExternalInput").ap()
    wu_d = nc.dram_tensor("wu", [D], bf16, kind="ExternalInput").ap()
    vecs_d = nc.dram_tensor("vecs", [3, KC, 128], f32, kind="ExternalInput").ap()
    out_d = nc.dram_tensor("out", [BPC, D, L], f32, kind="ExternalOutput").ap()
    upad_d = nc.dram_tensor("upad", [BPC * 2 * L], bf16).ap()  # internal scratch

    XSTR_B, XSTR_D = D * L, L  # element strides of x / out in DRAM

    with TileContext(nc) as tc:
        with (
            tc.tile_pool(name="const", bufs=1) as const,
            tc.tile_pool(name="xpool", bufs=12) as xpool,
            tc.tile_pool(name="hpool", bufs=12) as hpool,
            tc.tile_pool(name="tpool", bufs=2) as tpool,
            tc.tile_pool(name="opool", bufs=6) as opool,
            tc.tile_pool(name="upool", bufs=2) as upool,
            tc.tile_pool(name="pu", bufs=2, space="PSUM") as pu,
            tc.tile_pool(name="p3", bufs=3, space="PSUM") as p3,
            tc.tile_pool(name="p4", bufs=3, space="PSUM") as p4,
        ):
            # ---- constant tiles (DMA'd directly, already bf16 on host) ----
            whxT_r = const.tile([128, KC, D], bf16)    # [d' part | i_chunk | d]
            ct_r = const.tile([128, KC, D], bf16)      # [dh part | i_chunk | o]
            f_r = const.tile([128, D], bf16)           # [t' part | d]
            vecs_sb = const.tile([128, KC, 3], f32)    # b_h, bias', b_u
            wu_r = const.tile([128, KC], bf16)

            # minimal prologue: only what u(cb0) needs, so the PE starts
            # within a few microseconds of kernel entry. All param DMAs use
            # contiguous 128-elem lines (per-chunk) — a single strided DMA
            # here degrades to per-element descriptors and takes ~8us.
            for i in range(KC):
                nc.scalar.dma_start(
                    out=wu_r[:, i:i + 1],
                    in_=bass.AP(tensor=wu_d.tensor, offset=i * 128,
                                ap=[[1, 128]]),
                )
            for c in range(3):
                for i in range(KC):
                    nc.scalar.dma_start(
                        out=vecs_sb[:, i, c:c + 1],
                        in_=bass.AP(tensor=vecs_d.tensor,
                                    offset=(c * KC + i) * 128,
                                    ap=[[1, 128]]),
                    )
            # zero the upad scratch (pad halves stay zero forever)
            zt = const.tile([128, 2 * BPC], bf16)
            nc.vector.memset(zt[:], 0.0)
            nc.sync.dma_start(
                out=bass.AP(tensor=upad_d.tensor, offset=0,
                            ap=[[1, BPC * 2 * L]]),
                in_=zt[:],
            )

            def stage_whx():
                # whole [D, D] -> [128, KC, D] in two DMAs on gpsimd
                for half in range(2):
                    nc.gpsimd.dma_start(
                        out=whxT_r[:, half * (KC // 2):(half + 1) * (KC // 2), :],
                        in_=bass.AP(
                            tensor=whxT_d.tensor,
                            offset=half * (KC // 2) * 128 * D,
                            ap=[[D, 128], [128 * D, KC // 2], [1, D]],
                        ),
                    )

            def stage_f():
                nc.gpsimd.dma_start(out=f_r[:], in_=f_d)

            def stage_ct():
                for half in range(2):
                    nc.gpsimd.dma_start(
                        out=ct_r[:, half * (KC // 2):(half + 1) * (KC // 2), :],
                        in_=bass.AP(
                            tensor=ct_d.tensor,
                            offset=half * (KC // 2) * 128 * D,
                            ap=[[D, 128], [128 * D, KC // 2], [1, D]],
                        ),
                    )

            def load_x(cb):
                """DMA x tiles (bf16) for column block cb."""
                b0 = cb * NB
                xr = []
                for i in range(KC):
                    xt = xpool.tile([128, NCOL], bf16, tag="xt")
                    eng = (nc.sync, nc.gpsimd, nc.scalar)[i % 3]
                    eng.dma_start(
                        out=xt[:],
                        in_=bass.AP(
                            tensor=x_d.tensor,
                            offset=b0 * XSTR_B + i * 128 * XSTR_D,
                            ap=[[XSTR_D, 128], [XSTR_B, NB], [1, L]],
                        ),
                    )
                    xr.append(xt)
                return xr

            def compute_u(cb, xr):
                """u = relu(W_u @ x + b_u) -> upad scratch -> Toeplitz tile."""
                psu = pu.tile([1, NCOL], f32, tag="pu")
                for i in range(KC):
                    nc.tensor.matmul(psu[:], wu_r[:, i:i + 1], xr[i][:],
                                     start=(i == 0), stop=(i == KC - 1))
                u_sb = upool.tile([1, NCOL], bf16, tag="u")
                # u = relu(psu * 1 + b_u)
                nc.scalar.activation(u_sb[:], psu[:],
                                     mybir.ActivationFunctionType.Relu,
                                     bias=vecs_sb[0:1, 0, 2:3])
                nc.scalar.dma_start(
                    out=bass.AP(tensor=upad_d.tensor,
                                offset=cb * NB * 2 * L + L,
                                ap=[[2 * L, NB], [1, L]]),
                    in_=u_sb[:],
                )
                t_r = tpool.tile([128, NCOL], bf16, tag="tr")
                nc.scalar.dma_start(
                    out=t_r[:],
                    in_=bass.AP(tensor=upad_d.tensor,
                                offset=cb * NB * 2 * L + 1,
                                ap=[[1, 128], [2 * L, NB], [1, L]]),
                )
                return t_r

            def step34(cb, xr, t_r):
                b0 = cb * NB
                hs = []
                for j in range(KC):
                    ps3 = p3.tile([128, NCOL], f32, tag="ps3")
                    for i in range(KC):
                        nc.tensor.matmul(ps3[:], whxT_r[:, i, j * 128:(j + 1) * 128],
                                         xr[i][:], start=(i == 0), stop=False)
                    nc.tensor.matmul(ps3[:], f_r[:, j * 128:(j + 1) * 128], t_r[:],
                                     start=False, stop=True)
                    hj = hpool.tile([128, NCOL], bf16, tag="h")
                    nc.scalar.activation(hj[:], ps3[:],
                                         mybir.ActivationFunctionType.Relu,
                                         bias=vecs_sb[:, j, 0:1])
                    hs.append(hj)
                for j in range(KC):
                    ps4 = p4.tile([128, NCOL], f32, tag="ps4")
                    for i in range(KC):
                        nc.tensor.matmul(ps4[:], ct_r[:, i, j * 128:(j + 1) * 128],
                                         hs[i][:], start=(i == 0), stop=(i == KC - 1))
                    oj = opool.tile([128, NCOL], f32, tag="o")
                    nc.vector.tensor_scalar_add(oj[:], ps4[:], vecs_sb[:, j, 1:2])
                    oeng = nc.sync if j % 2 == 0 else nc.gpsimd
                    oeng.dma_start(
                        out=bass.AP(
                            tensor=out_d.tensor,
                            offset=b0 * XSTR_B + j * 128 * XSTR_D,
                            ap=[[XSTR_D, 128], [XSTR_B, NB], [1, L]],
                        ),
                        in_=oj[:],
                    )

            # software pipeline: u-chain for block cb+1 runs while step3/4
            # of block cb keeps the PE busy (Toeplitz tile is ready one
            # block ahead, so its matmul never stalls the PE).
            xr_cur = load_x(0)
            stage_whx()
            stage_f()
            stage_ct()
            t_cur = compute_u(0, xr_cur)
            for cb in range(NCB):
                if cb + 1 < NCB:
                    xr_next = load_x(cb + 1)
                    t_next = compute_u(cb + 1, xr_next)
                else:
                    xr_next = t_next = None
                step34(cb, xr_cur, t_cur)
                xr_cur, t_cur = xr_next, t_next

    if not nc.is_finalized():
        nc.finalize()
    return nc


def _get_nc():
    global _NC_CACHE
    if _NC_CACHE is None:
        _NC_CACHE = _build_nc()
    return _NC_CACHE


def _ensure_ntff_hook():
    """Register the NTFF profile hook if the deployment lacks antenv.axon_hooks."""
    import sys
    import types
    try:
        from antenv.axon_hooks import get_axon_ntff_profile_hook  # noqa: F401
        return
    except ImportError:
        pass
    try:
        from trn_agent_boot.trn_boot import _ntff_profile_via_ctypes
        hook = _ntff_profile_via_ctypes("/opt/axon/libaxon_pjrt.so")
        mod = types.ModuleType("antenv.axon_hooks")
        mod.get_axon_ntff_profile_hook = lambda: hook
        mod.set_axon_ntff_profile_hook = lambda h: None
        import antenv
        sys.modules["antenv.axon_hooks"] = mod
        antenv.axon_hooks = mod
    except Exception:
        pass


def kernel(x, W_u, b_u, W_h, b_h, conv_w, conv_b, bn_gamma, bn_beta, bn_mean,
           bn_var):
    global LAST_EXEC_NS
    bf16 = ml_dtypes.bfloat16
    x = np.ascontiguousarray(np.asarray(x, dtype=np.float32)).astype(bf16)
    W_u = np.asarray(W_u, dtype=np.float64)
    b_u = np.asarray(b_u, dtype=np.float64)
    W_h = np.asarray(W_h, dtype=np.float64)
    b_h = np.asarray(b_h, dtype=np.float64)
    conv_w = np.asarray(conv_w, dtype=np.float64)
    conv_b = np.asarray(conv_b, dtype=np.float64)
    bn_gamma = np.asarray(bn_gamma, dtype=np.float64)
    bn_beta = np.asarray(bn_beta, dtype=np.float64)
    bn_mean = np.asarray(bn_mean, dtype=np.float64)
    bn_var = np.asarray(bn_var, dtype=np.float64)
    assert x.shape == (B, D, L)

    H = _impulse_response().astype(np.float64)  # [D, L]

    # host folds (O(params) only)
    F = (W_h[:, :D] @ H).T[::-1, :]                      # [L, D], row-flipped
    whxT = np.ascontiguousarray(W_h[:, D:].T)            # [D(d'), D(d)]
    inv = bn_gamma / np.sqrt(bn_var + BN_EPS)
    ct = np.ascontiguousarray((conv_w[:, :, 0] * inv[:, None]).T)  # [dh, o]
    bias2 = (conv_b - bn_mean) * inv + bn_beta
    # [3, KC, 128]: contiguous 128-runs per (column, chunk) for fast DMA
    vecs = np.stack([b_h, bias2, np.full(D, b_u[0])], axis=0).reshape(3, KC, 128)

    nc = _get_nc()
    shared = {
        "whxT": whxT.astype(np.float32).astype(bf16),
        "ct": ct.astype(np.float32).astype(bf16),
        "fmat": np.ascontiguousarray(F).astype(np.float32).astype(bf16),
        "wu": W_u[0].astype(np.float32).astype(bf16),
        "vecs": vecs.astype(np.float32),
    }
    in_maps = []
    for c in range(NCORES):
        m = dict(shared)
        m["x"] = x[c * BPC:(c + 1) * BPC]
        in_maps.append(m)

    if TRACE:
        _ensure_ntff_hook()
    res = run_bass_kernel_spmd(nc, in_maps, list(range(NCORES)), trace=TRACE)
    LAST_EXEC_NS = res.exec_time_ns
    out = np.concatenate([res.results[c]["out"] for c in range(NCORES)], axis=0)
    return out


# revision 11
# speedup vs baseline: 1.0729x; 1.0729x over previous
"""LMU kernel for Trainium2, 8-core data-parallel.

Math (per batch b, with x[b] in [D, L] layout):
  u[b]    = relu(W_u @ x[b] + b_u)                              [1, L]
  m[b]    = H @ Toep(u[b])        (causal conv via Toeplitz)    [D, L]
  h[b]    = relu(W_h[:, :D] @ m[b] + W_h[:, D:] @ x[b] + b_h)   [D, L]
  y[b]    = BN(conv_w @ h[b] + conv_b)                          [D, L]

Device-side folds (host precomputes, O(params) only):
  F      = (W_h[:, :D] @ H).T, row-flipped  -> single K=128 contraction
           against the (flipped) Toeplitz of u
  C'     = (inv * conv_w).T, bias' = (conv_b - mean) * inv + beta   (BN fold)

All matmul operands are bf16 (host-cast), so LDWEIGHTS hides fully under
the 1 col/cycle stream and no on-device casts are needed anywhere.
Batch dim sharded 8 ways; params replicated.
"""

import os
import numpy as np
import ml_dtypes

import concourse.bass as bass
import concourse.mybir as mybir
from concourse import bacc
from concourse.tile import TileContext
from concourse.bass_utils import run_bass_kernel_spmd

B, D, L = 256, 768, 128
NCORES = 8
BPC = B // NCORES          # batches per core
NB = 4                     # batches per column block
NCB = BPC // NB            # column blocks per core
NCOL = NB * L              # 512 columns per block
KC = D // 128              # 6 chunks of 128 over the D dim
THETA = 128.0
BN_EPS = 1e-5

TRACE = False
LAST_EXEC_NS = None

_H_CACHE = None
_NC_CACHE = None


def _impulse_response():
    """Replicates the reference's H = impulse response [D, L], on CPU."""
    global _H_CACHE
    if _H_CACHE is not None:
        return _H_CACHE
    import jax
    import jax.numpy as jnp
    from jax.scipy.linalg import expm

    cpu = jax.devices("cpu")[0]
    with jax.default_device(cpu):
        Q = np.arange(D, dtype=np.float32)
        R = ((2.0 * Q + 1.0) / THETA)[:, None]
        i, j = np.meshgrid(Q, Q, indexing="ij")
        A = (np.where(i < j, -1.0, (-1.0) ** (i - j + 1)).astype(np.float32)) * R
        Bm = (((-1.0) ** Q)[:, None]).astype(np.float32) * R
        Maug = np.zeros((D + 1, D + 1), dtype=np.float32)
        Maug[:D, :D] = A
        Maug[:D, D:] = Bm
        E = expm(jnp.asarray(Maug))
        Ad = E[:D, :D]
        Bd = E[:D, D:]

        def step(Apow, _):
            return Ad @ Apow, (Apow @ Bd)[:, 0]

        _, H = jax.lax.scan(step, jnp.eye(D, dtype=jnp.float32), None, length=L)
        _H_CACHE = np.asarray(H).T.astype(np.float32)  # [D, L]
    return _H_CACHE


def _build_nc():
    """Builds the (static) 8-core SPMD Bass program."""
    f32 = mybir.dt.float32
    bf16 = mybir.dt.bfloat16
    nc = bacc.Bacc("TRN2", target_bir_lowering=False, debug=False, num_devices=NCORES)

    x_d = nc.dram_tensor("x", [BPC, D, L], bf16, kind="ExternalInput").ap()
    whxT_d = nc.dram_tensor("whxT", [D, D], bf16, kind="ExternalInput").ap()
    ct_d = nc.dram_tensor("ct", [D, D], bf16, kind="ExternalInput").ap()
    f_d = nc.dram_tensor("fmat", [L, D], bf16, kind="ExternalInput").ap()
    wu_d = nc.dram_tensor("wu", [128, KC], bf16, kind="ExternalInput").ap()
    vecs_d = nc.dram_tensor("vecs", [128, KC, 3], f32, kind="ExternalInput").ap()
    out_d = nc.dram_tensor("out", [BPC, D, L], f32, kind="ExternalOutput").ap()
    upad_d = nc.dram_tensor("upad", [BPC * 2 * L], bf16).ap()  # internal scratch

    XSTR_B, XSTR_D = D * L, L  # element strides of x / out in DRAM

    with TileContext(nc) as tc:
        with (
            tc.tile_pool(name="const", bufs=1) as const,
            tc.tile_pool(name="xpool", bufs=12) as xpool,
            tc.tile_pool(name="hpool", bufs=12) as hpool,
            tc.tile_pool(name="tpool", bufs=2) as tpool,
            tc.tile_pool(name="opool", bufs=6) as opool,
            tc.tile_pool(name="upool", bufs=2) as upool,
            tc.tile_pool(name="pu", bufs=2, space="PSUM") as pu,
            tc.tile_pool(name="p3", bufs=3, space="PSUM") as p3,
            tc.tile_pool(name="p4", bufs=3, space="PSUM") as p4,
        ):
            # ---- constant tiles (DMA'd directly, already bf16 on host) ----
            whxT_r = const.tile([128, KC, D], bf16)    # [d' part | i_chunk | d]
            ct_r = const.tile([128, KC, D], bf16)      # [dh part | i_chunk | o]
            f_r = const.tile([128, D], bf16)           # [t' part | d]
            vecs_sb = const.tile([128, KC, 3], f32)    # b_h, bias', b_u
            wu_r = const.tile([128, KC], bf16)

            # minimal prologue: only what u(cb0) needs, so the PE starts
            # within a few microseconds of kernel entry. Param DRAM layout is
            # partition-major so each partition's free-dim run is contiguous
            # (per-element descriptor lines here cost ~10ns each and can add
            # ~8us to the critical path otherwise).
            nc.scalar.dma_start(out=wu_r[:], in_=wu_d)
            nc.scalar.dma_start(out=vecs_sb[:], in_=vecs_d)
            # zero the upad scratch (pad halves stay zero forever)
            zt = const.tile([128, 2 * BPC], bf16)
            nc.vector.memset(zt[:], 0.0)
            nc.sync.dma_start(
                out=bass.AP(tensor=upad_d.tensor, offset=0,
                            ap=[[1, BPC * 2 * L]]),
                in_=zt[:],
            )

            def stage_whx():
                # whole [D, D] -> [128, KC, D] in two DMAs on gpsimd
                for half in range(2):
                    nc.gpsimd.dma_start(
                        out=whxT_r[:, half * (KC // 2):(half + 1) * (KC // 2), :],
                        in_=bass.AP(
                            tensor=whxT_d.tensor,
                            offset=half * (KC // 2) * 128 * D,
                            ap=[[D, 128], [128 * D, KC // 2], [1, D]],
                        ),
                    )

            def stage_f():
                nc.gpsimd.dma_start(out=f_r[:], in_=f_d)

            def stage_ct():
                for half in range(2):
                    nc.gpsimd.dma_start(
                        out=ct_r[:, half * (KC // 2):(half + 1) * (KC // 2), :],
                        in_=bass.AP(
                            tensor=ct_d.tensor,
                            offset=half * (KC // 2) * 128 * D,
                            ap=[[D, 128], [128 * D, KC // 2], [1, D]],
                        ),
                    )

            def load_x(cb):
                """DMA x tiles (bf16) for column block cb."""
                b0 = cb * NB
                xr = []
                for i in range(KC):
                    xt = xpool.tile([128, NCOL], bf16, tag="xt")
                    eng = (nc.sync, nc.gpsimd, nc.scalar)[i % 3]
                    eng.dma_start(
                        out=xt[:],
                        in_=bass.AP(
                            tensor=x_d.tensor,
                            offset=b0 * XSTR_B + i * 128 * XSTR_D,
                            ap=[[XSTR_D, 128], [XSTR_B, NB], [1, L]],
                        ),
                    )
                    xr.append(xt)
                return xr

            def compute_u(cb, xr):
                """u = relu(W_u @ x + b_u) -> upad scratch -> Toeplitz tile."""
                psu = pu.tile([1, NCOL], f32, tag="pu")
                for i in range(KC):
                    nc.tensor.matmul(psu[:], wu_r[:, i:i + 1], xr[i][:],
                                     start=(i == 0), stop=(i == KC - 1))
                u_sb = upool.tile([1, NCOL], bf16, tag="u")
                # u = relu(psu * 1 + b_u)
                nc.scalar.activation(u_sb[:], psu[:],
                                     mybir.ActivationFunctionType.Relu,
                                     bias=vecs_sb[0:1, 0, 2:3])
                nc.scalar.dma_start(
                    out=bass.AP(tensor=upad_d.tensor,
                                offset=cb * NB * 2 * L + L,
                                ap=[[2 * L, NB], [1, L]]),
                    in_=u_sb[:],
                )
                t_r = tpool.tile([128, NCOL], bf16, tag="tr")
                nc.scalar.dma_start(
                    out=t_r[:],
                    in_=bass.AP(tensor=upad_d.tensor,
                                offset=cb * NB * 2 * L + 1,
                                ap=[[1, 128], [2 * L, NB], [1, L]]),
                )
                return t_r

            def step34(cb, xr, t_r):
                b0 = cb * NB
                hs = []
                for j in range(KC):
                    ps3 = p3.tile([128, NCOL], f32, tag="ps3")
                    for i in range(KC):
                        nc.tensor.matmul(ps3[:], whxT_r[:, i, j * 128:(j + 1) * 128],
                                         xr[i][:], start=(i == 0), stop=False)
                    nc.tensor.matmul(ps3[:], f_r[:, j * 128:(j + 1) * 128], t_r[:],
                                     start=False, stop=True)
                    hj = hpool.tile([128, NCOL], bf16, tag="h")
                    nc.scalar.activation(hj[:], ps3[:],
                                         mybir.ActivationFunctionType.Relu,
                                         bias=vecs_sb[:, j, 0:1])
                    hs.append(hj)
                for j in range(KC):
                    ps4 = p4.tile([128, NCOL], f32, tag="ps4")
                    for i in range(KC):
                        nc.tensor.matmul(ps4[:], ct_r[:, i, j * 128:(j + 1) * 128],
                                         hs[i][:], start=(i == 0), stop=(i == KC - 1))
                    oj = opool.tile([128, NCOL], f32, tag="o")
                    nc.vector.tensor_scalar_add(oj[:], ps4[:], vecs_sb[:, j, 1:2])
                    oeng = nc.sync if j % 2 == 0 else nc.gpsimd
                    oeng.dma_start(
                        out=bass.AP(
                            tensor=out_d.tensor,
                            offset=b0 * XSTR_B + j * 128 * XSTR_D,
                            ap=[[XSTR_D, 128], [XSTR_B, NB], [1, L]],
                        ),
                        in_=oj[:],
                    )

            # software pipeline: u-chain for block cb+1 runs while step3/4
            # of block cb keeps the PE busy (Toeplitz tile is ready one
            # block ahead, so its matmul never stalls the PE).
            xr_cur = load_x(0)
            stage_whx()
            stage_f()
            stage_ct()
            t_cur = compute_u(0, xr_cur)
            for cb in range(NCB):
                if cb + 1 < NCB:
                    xr_next = load_x(cb + 1)
                    t_next = compute_u(cb + 1, xr_next)
                else:
                    xr_next = t_next = None
                step34(cb, xr_cur, t_cur)
                xr_cur, t_cur = xr_next, t_next

    if not nc.is_finalized():
        nc.finalize()
    return nc


def _get_nc():
    global _NC_CACHE
    if _NC_CACHE is None:
        _NC_CACHE = _build_nc()
    return _NC_CACHE


def _ensure_ntff_hook():
    """Register the NTFF profile hook if the deployment lacks antenv.axon_hooks."""
    import sys
    import types
    try:
        from antenv.axon_hooks import get_axon_ntff_profile_hook  # noqa: F401
        return
    except ImportError:
        pass
    try:
        from trn_agent_boot.trn_boot import _ntff_profile_via_ctypes
        hook = _ntff_profile_via_ctypes("/opt/axon/libaxon_pjrt.so")
        mod = types.ModuleType("antenv.axon_hooks")
        mod.get_axon_ntff_profile_hook = lambda: hook
        mod.set_axon_ntff_profile_hook = lambda h: None
        import antenv
        sys.modules["antenv.axon_hooks"] = mod
        antenv.axon_hooks = mod
    except Exception:
        pass


def kernel(x, W_u, b_u, W_h, b_h, conv_w, conv_b, bn_gamma, bn_beta, bn_mean,
           bn_var):
    global LAST_EXEC_NS
    bf16 = ml_dtypes.bfloat16
    x = np.ascontiguousarray(np.asarray(x, dtype=np.float32)).astype(bf16)
    W_u = np.asarray(W_u, dtype=np.float64)
    b_u = np.asarray(b_u, dtype=np.float64)
    W_h = np.asarray(W_h, dtype=np.float64)
    b_h = np.asarray(b_h, dtype=np.float64)
    conv_w = np.asarray(conv_w, dtype=np.float64)
    conv_b = np.asarray(conv_b, dtype=np.float64)
    bn_gamma = np.asarray(bn_gamma, dtype=np.float64)
    bn_beta = np.asarray(bn_beta, dtype=np.float64)
    bn_mean = np.asarray(bn_mean, dtype=np.float64)
    bn_var = np.asarray(bn_var, dtype=np.float64)
    assert x.shape == (B, D, L)

    H = _impulse_response().astype(np.float64)  # [D, L]

    # host folds (O(params) only)
    F = (W_h[:, :D] @ H).T[::-1, :]                      # [L, D], row-flipped
    whxT = np.ascontiguousarray(W_h[:, D:].T)            # [D(d'), D(d)]
    inv = bn_gamma / np.sqrt(bn_var + BN_EPS)
    ct = np.ascontiguousarray((conv_w[:, :, 0] * inv[:, None]).T)  # [dh, o]
    bias2 = (conv_b - bn_mean) * inv + bn_beta
    # [128, KC, 3]: partition-major, contiguous free-dim run per partition
    vecs = np.stack([b_h, bias2, np.full(D, b_u[0])], axis=1)  # [D, 3]
    vecs = np.ascontiguousarray(vecs.reshape(KC, 128, 3).transpose(1, 0, 2))

    nc = _get_nc()
    shared = {
        "whxT": whxT.astype(np.float32).astype(bf16),
        "ct": ct.astype(np.float32).astype(bf16),
        "fmat": np.ascontiguousarray(F).astype(np.float32).astype(bf16),
        "wu": np.ascontiguousarray(
            W_u[0].astype(np.float32).astype(bf16).reshape(KC, 128).T),
        "vecs": vecs.astype(np.float32),
    }
    in_maps = []
    for c in range(NCORES):
        m = dict(shared)
        m["x"] = x[c * BPC:(c + 1) * BPC]
        in_maps.append(m)

    if TRACE:
        _ensure_ntff_hook()
    res = run_bass_kernel_spmd(nc, in_maps, list(range(NCORES)), trace=TRACE)
    LAST_EXEC_NS = res.exec_time_ns
    out = np.concatenate([res.results[c]["out"] for c in range(NCORES)], axis=0)
    return out


# revision 16
# speedup vs baseline: 1.0886x; 1.0146x over previous
"""LMU kernel for Trainium2, 8-core data-parallel.

Math (per batch b, with x[b] in [D, L] layout):
  u[b]    = relu(W_u @ x[b] + b_u)                              [1, L]
  m[b]    = H @ Toep(u[b])        (causal conv via Toeplitz)    [D, L]
  h[b]    = relu(W_h[:, :D] @ m[b] + W_h[:, D:] @ x[b] + b_h)   [D, L]
  y[b]    = BN(conv_w @ h[b] + conv_b)                          [D, L]

Device-side folds (host precomputes, O(params) only):
  F      = (W_h[:, :D] @ H).T, row-flipped  -> single K=128 contraction
           against the (flipped) Toeplitz of u
  C'     = (inv * conv_w).T, bias' = (conv_b - mean) * inv + beta   (BN fold)

All matmul operands are bf16 (host-cast), so LDWEIGHTS hides fully under
the 1 col/cycle stream and no on-device casts are needed anywhere.
Batch dim sharded 8 ways; params replicated.
"""

import os
import numpy as np
import ml_dtypes

import concourse.bass as bass
import concourse.mybir as mybir
from concourse import bacc
from concourse.tile import TileContext
from concourse.bass_utils import run_bass_kernel_spmd

B, D, L = 256, 768, 128
NCORES = 8
BPC = B // NCORES          # batches per core
NB = 4                     # batches per column block
NCB = BPC // NB            # column blocks per core
NCOL = NB * L              # 512 columns per block
KC = D // 128              # 6 chunks of 128 over the D dim
THETA = 128.0
BN_EPS = 1e-5

TRACE = False
LAST_EXEC_NS = None

_H_CACHE = None
_NC_CACHE = None


def _impulse_response():
    """Replicates the reference's H = impulse response [D, L], on CPU."""
    global _H_CACHE
    if _H_CACHE is not None:
        return _H_CACHE
    import jax
    import jax.numpy as jnp
    from jax.scipy.linalg import expm

    cpu = jax.devices("cpu")[0]
    with jax.default_device(cpu):
        Q = np.arange(D, dtype=np.float32)
        R = ((2.0 * Q + 1.0) / THETA)[:, None]
        i, j = np.meshgrid(Q, Q, indexing="ij")
        A = (np.where(i < j, -1.0, (-1.0) ** (i - j + 1)).astype(np.float32)) * R
        Bm = (((-1.0) ** Q)[:, None]).astype(np.float32) * R
        Maug = np.zeros((D + 1, D + 1), dtype=np.float32)
        Maug[:D, :D] = A
        Maug[:D, D:] = Bm
        E = expm(jnp.asarray(Maug))
        Ad = E[:D, :D]
        Bd = E[:D, D:]

        def step(Apow, _):
            return Ad @ Apow, (Apow @ Bd)[:, 0]

        _, H = jax.lax.scan(step, jnp.eye(D, dtype=jnp.float32), None, length=L)
        _H_CACHE = np.asarray(H).T.astype(np.float32)  # [D, L]
    return _H_CACHE


def _build_nc():
    """Builds the (static) 8-core SPMD Bass program."""
    f32 = mybir.dt.float32
    bf16 = mybir.dt.bfloat16
    nc = bacc.Bacc("TRN2", target_bir_lowering=False, debug=False, num_devices=NCORES)

    x_d = nc.dram_tensor("x", [BPC, D, L], bf16, kind="ExternalInput").ap()
    whxT_d = nc.dram_tensor("whxT", [D, D], bf16, kind="ExternalInput").ap()
    ct_d = nc.dram_tensor("ct", [D, D], bf16, kind="ExternalInput").ap()
    f_d = nc.dram_tensor("fmat", [L, D], bf16, kind="ExternalInput").ap()
    wu_d = nc.dram_tensor("wu", [128, KC], bf16, kind="ExternalInput").ap()
    vecs_d = nc.dram_tensor("vecs", [128, KC, 3], f32, kind="ExternalInput").ap()
    out_d = nc.dram_tensor("out", [BPC, D, L], bf16, kind="ExternalOutput").ap()
    upad_d = nc.dram_tensor("upad", [BPC * 2 * L], bf16).ap()  # internal scratch

    XSTR_B, XSTR_D = D * L, L  # element strides of x / out in DRAM

    with TileContext(nc) as tc:
        with (
            tc.tile_pool(name="const", bufs=1) as const,
            tc.tile_pool(name="xpool", bufs=12) as xpool,
            tc.tile_pool(name="hpool", bufs=12) as hpool,
            tc.tile_pool(name="tpool", bufs=2) as tpool,
            tc.tile_pool(name="opool", bufs=6) as opool,
            tc.tile_pool(name="upool", bufs=2) as upool,
            tc.tile_pool(name="pu", bufs=2, space="PSUM") as pu,
            tc.tile_pool(name="p3", bufs=3, space="PSUM") as p3,
            tc.tile_pool(name="p4", bufs=3, space="PSUM") as p4,
        ):
            # ---- constant tiles (DMA'd directly, already bf16 on host) ----
            whxT_r = const.tile([128, KC, D], bf16)    # [d' part | i_chunk | d]
            ct_r = const.tile([128, KC, D], bf16)      # [dh part | i_chunk | o]
            f_r = const.tile([128, D], bf16)           # [t' part | d]
            vecs_sb = const.tile([128, KC, 3], f32)    # b_h, bias', b_u
            wu_r = const.tile([128, KC], bf16)

            # minimal prologue: only what u(cb0) needs, so the PE starts
            # within a few microseconds of kernel entry. Param DRAM layout is
            # partition-major so each partition's free-dim run is contiguous
            # (per-element descriptor lines here cost ~10ns each and can add
            # ~8us to the critical path otherwise).
            nc.scalar.dma_start(out=wu_r[:], in_=wu_d)
            nc.scalar.dma_start(out=vecs_sb[:], in_=vecs_d)
            # zero the upad scratch (pad halves stay zero forever)
            zt = const.tile([128, 2 * BPC], bf16)
            nc.vector.memset(zt[:], 0.0)
            nc.sync.dma_start(
                out=bass.AP(tensor=upad_d.tensor, offset=0,
                            ap=[[1, BPC * 2 * L]]),
                in_=zt[:],
            )

            def stage_whx():
                # per-chunk DMAs, alternating queues, to avoid head-of-line
                # blocking the x tile loads (each chunk is ~196KB / ~1us)
                for i in range(KC):
                    eng = nc.sync if i % 2 == 0 else nc.gpsimd
                    eng.dma_start(
                        out=whxT_r[:, i, :],
                        in_=bass.AP(tensor=whxT_d.tensor, offset=i * 128 * D,
                                    ap=[[D, 128], [1, D]]),
                    )

            def stage_f():
                nc.sync.dma_start(out=f_r[:], in_=f_d)

            def stage_ct():
                for i in range(KC):
                    eng = nc.sync if i % 2 == 0 else nc.gpsimd
                    eng.dma_start(
                        out=ct_r[:, i, :],
                        in_=bass.AP(tensor=ct_d.tensor, offset=i * 128 * D,
                                    ap=[[D, 128], [1, D]]),
                    )

            def load_x(cb):
                """DMA x tiles (bf16) for column block cb."""
                b0 = cb * NB
                xr = []
                for i in range(KC):
                    xt = xpool.tile([128, NCOL], bf16, tag="xt")
                    eng = (nc.sync, nc.gpsimd, nc.scalar)[i % 3]
                    eng.dma_start(
                        out=xt[:],
                        in_=bass.AP(
                            tensor=x_d.tensor,
                            offset=b0 * XSTR_B + i * 128 * XSTR_D,
                            ap=[[XSTR_D, 128], [XSTR_B, NB], [1, L]],
                        ),
                    )
                    xr.append(xt)
                return xr

            def compute_u(cb, xr):
                """u = relu(W_u @ x + b_u) -> upad scratch -> Toeplitz tile."""
                psu = pu.tile([1, NCOL], f32, tag="pu")
                for i in range(KC):
                    nc.tensor.matmul(psu[:], wu_r[:, i:i + 1], xr[i][:],
                                     start=(i == 0), stop=(i == KC - 1))
                u_sb = upool.tile([1, NCOL], bf16, tag="u")
                # u = relu(psu * 1 + b_u)
                nc.scalar.activation(u_sb[:], psu[:],
                                     mybir.ActivationFunctionType.Relu,
                                     bias=vecs_sb[0:1, 0, 2:3])
                nc.scalar.dma_start(
                    out=bass.AP(tensor=upad_d.tensor,
                                offset=cb * NB * 2 * L + L,
                                ap=[[2 * L, NB], [1, L]]),
                    in_=u_sb[:],
                )
                t_r = tpool.tile([128, NCOL], bf16, tag="tr")
                nc.scalar.dma_start(
                    out=t_r[:],
                    in_=bass.AP(tensor=upad_d.tensor,
                                offset=cb * NB * 2 * L + 1,
                                ap=[[1, 128], [2 * L, NB], [1, L]]),
                )
                return t_r

            def step34(cb, xr, t_r):
                b0 = cb * NB
                hs = []
                for j in range(KC):
                    ps3 = p3.tile([128, NCOL], f32, tag="ps3")
                    for i in range(KC):
                        nc.tensor.matmul(ps3[:], whxT_r[:, i, j * 128:(j + 1) * 128],
                                         xr[i][:], start=(i == 0), stop=False)
                    nc.tensor.matmul(ps3[:], f_r[:, j * 128:(j + 1) * 128], t_r[:],
                                     start=False, stop=True)
                    hj = hpool.tile([128, NCOL], bf16, tag="h")
                    nc.scalar.activation(hj[:], ps3[:],
                                         mybir.ActivationFunctionType.Relu,
                                         bias=vecs_sb[:, j, 0:1])
                    hs.append(hj)
                for j in range(KC):
                    ps4 = p4.tile([128, NCOL], f32, tag="ps4")
                    for i in range(KC):
                        nc.tensor.matmul(ps4[:], ct_r[:, i, j * 128:(j + 1) * 128],
                                         hs[i][:], start=(i == 0), stop=(i == KC - 1))
                    oj = opool.tile([128, NCOL], bf16, tag="o")
                    nc.vector.tensor_scalar_add(oj[:], ps4[:], vecs_sb[:, j, 1:2])
                    oeng = nc.sync if j % 2 == 0 else nc.gpsimd
                    oeng.dma_start(
                        out=bass.AP(
                            tensor=out_d.tensor,
                            offset=b0 * XSTR_B + j * 128 * XSTR_D,
                            ap=[[XSTR_D, 128], [XSTR_B, NB], [1, L]],
                        ),
                        in_=oj[:],
                    )

            # software pipeline: u-chain for block cb+1 runs while step3/4
            # of block cb keeps the PE busy (Toeplitz tile is ready one
            # block ahead, so its matmul never stalls the PE).
            with tc.high_priority():
                xr_cur = load_x(0)
            stage_whx()
            stage_f()
            t_cur = compute_u(0, xr_cur)
            stage_ct()
            for cb in range(NCB):
                if cb + 1 < NCB:
                    xr_next = load_x(cb + 1)
                    t_next = compute_u(cb + 1, xr_next)
                else:
                    xr_next = t_next = None
                step34(cb, xr_cur, t_cur)
                xr_cur, t_cur = xr_next, t_next

    if not nc.is_finalized():
        nc.finalize()
    return nc


def _get_nc():
    global _NC_CACHE
    if _NC_CACHE is None:
        _NC_CACHE = _build_nc()
    return _NC_CACHE


def _ensure_ntff_hook():
    """Register the NTFF profile hook if the deployment lacks antenv.axon_hooks."""
    import sys
    import types
    try:
        from antenv.axon_hooks import get_axon_ntff_profile_hook  # noqa: F401
        return
    except ImportError:
        pass
    try:
        from trn_agent_boot.trn_boot import _ntff_profile_via_ctypes
        hook = _ntff_profile_via_ctypes("/opt/axon/libaxon_pjrt.so")
        mod = types.ModuleType("antenv.axon_hooks")
        mod.get_axon_ntff_profile_hook = lambda: hook
        mod.set_axon_ntff_profile_hook = lambda h: None
        import antenv
        sys.modules["antenv.axon_hooks"] = mod
        antenv.axon_hooks = mod
    except Exception:
        pass


def kernel(x, W_u, b_u, W_h, b_h, conv_w, conv_b, bn_gamma, bn_beta, bn_mean,
           bn_var):
    global LAST_EXEC_NS
    bf16 = ml_dtypes.bfloat16
    x = np.ascontiguousarray(np.asarray(x, dtype=np.float32)).astype(bf16)
    W_u = np.asarray(W_u, dtype=np.float64)
    b_u = np.asarray(b_u, dtype=np.float64)
    W_h = np.asarray(W_h, dtype=np.float64)
    b_h = np.asarray(b_h, dtype=np.float64)
    conv_w = np.asarray(conv_w, dtype=np.float64)
    conv_b = np.asarray(conv_b, dtype=np.float64)
    bn_gamma = np.asarray(bn_gamma, dtype=np.float64)
    bn_beta = np.asarray(bn_beta, dtype=np.float64)
    bn_mean = np.asarray(bn_mean, dtype=np.float64)
    bn_var = np.asarray(bn_var, dtype=np.float64)
    assert x.shape == (B, D, L)

    H = _impulse_response().astype(np.float64)  # [D, L]

    # host folds (O(params) only)
    F = (W_h[:, :D] @ H).T[::-1, :]                      # [L, D], row-flipped
    whxT = np.ascontiguousarray(W_h[:, D:].T)            # [D(d'), D(d)]
    inv = bn_gamma / np.sqrt(bn_var + BN_EPS)
    ct = np.ascontiguousarray((conv_w[:, :, 0] * inv[:, None]).T)  # [dh, o]
    bias2 = (conv_b - bn_mean) * inv + bn_beta
    # [128, KC, 3]: partition-major, contiguous free-dim run per partition
    vecs = np.stack([b_h, bias2, np.full(D, b_u[0])], axis=1)  # [D, 3]
    vecs = np.ascontiguousarray(vecs.reshape(KC, 128, 3).transpose(1, 0, 2))

    nc = _get_nc()
    shared = {
        "whxT": whxT.astype(np.float32).astype(bf16),
        "ct": ct.astype(np.float32).astype(bf16),
        "fmat": np.ascontiguousarray(F).astype(np.float32).astype(bf16),
        "wu": np.ascontiguousarray(
            W_u[0].astype(np.float32).astype(bf16).reshape(KC, 128).T),
        "vecs": vecs.astype(np.float32),
    }
    in_maps = []
    for c in range(NCORES):
        m = dict(shared)
        m["x"] = x[c * BPC:(c + 1) * BPC]
        in_maps.append(m)

    if TRACE:
        _ensure_ntff_hook()
    res = run_bass_kernel_spmd(nc, in_maps, list(range(NCORES)), trace=TRACE)
    LAST_EXEC_NS = res.exec_time_ns
    out = np.concatenate([res.results[c]["out"] for c in range(NCORES)], axis=0)
    return out.astype(np.float32)
